# revision 55
# baseline (speedup 1.0000x reference)
"""Local (banded) attention kernel for Trainium2, sharded over 8 NeuronCores.

Sharding: core c handles batch b=c//4 and heads 4*(c%4)..4*(c%4)+3.
Host pre-transposes x and weight slices; device does QKV projection,
banded attention (window 128 -> only tile-diagonal +/-1 blocks), and the
per-core slice of the output projection. Host sums the 4 partial outputs
per batch and adds the output bias.

Mixed precision: q/k projections run in fp8e4m3 with DoubleRow perf mode
(two contraction sub-tiles per pass); v / energy / AV / output projection
run in bf16.  The schedule splits per-tile work into two pipeline stages
(attention, then projection) offset by one tile so no engine queue has a
head-of-line dependency on the previous tile's downstream work.
"""

import ml_dtypes
import numpy as np

import concourse.bass as bass
import concourse.mybir as mybir
from concourse import bacc
from concourse.tile import TileContext
from concourse.bass_utils import run_bass_kernel_spmd
from concourse.masks import make_identity

B, N, E, H, DH, WIN = 2, 2048, 1024, 16, 64, 128
HPC = 4              # heads per core
SL = HPC * DH        # feature slice per core (256)
NT = N // 128        # 16 query/key tiles
F32 = mybir.dt.float32
BF16 = mybir.dt.bfloat16
FP8 = mybir.dt.float8e4
SCALE = 1.0 / 32.0   # 1/sqrt(E)
KO = E // 128        # 8 contraction tiles
NP_FP8 = ml_dtypes.float8_e4m3fn if hasattr(ml_dtypes, "float8_e4m3fn") else ml_dtypes.float8_e4m3
NP_BF16 = ml_dtypes.bfloat16

_CACHED_NC = None

# build-time experiment knobs
CFG = {
    "energy_dr": False,   # fp8 DoubleRow energy via d-split rearrange
    "dr_direct": True,    # projection writes d-split layout; DR energy in place
    "y_direct": False,
    "late_act": False,
    "tight_tail": False,
    "attT_act": 0,
    "qk_act": 0,
    "y_act": 1,
    "qk_pos": (0, 2, 4),
    "v_lead": 1,
    "strip_bufs": 16,
    "late_from": 99,
    "late_y_from": 12,
    "y_in_mm": True,
    "ps_e_bufs": 3,
    "ps_mm_bufs": 3,
    "ps_u_bufs": 1,
    "ps_t_bufs": 1,
    "ps_y_bufs": 0,
}


def _build_nc():
    nc = bacc.Bacc("TRN2", target_bir_lowering=False)
    DR = mybir.MatmulPerfMode.DoubleRow
    QKD = FP8 if (CFG["energy_dr"] or CFG["dr_direct"]) else BF16

    xT8_d = nc.dram_tensor("xT8", [E, N], FP8, kind="ExternalInput")
    xTb_d = nc.dram_tensor("xTb", [E, N], BF16, kind="ExternalInput")
    wq8_d = nc.dram_tensor("wq8", [E, SL], FP8, kind="ExternalInput")
    wk8_d = nc.dram_tensor("wk8", [E, SL], FP8, kind="ExternalInput")
    wvb_d = nc.dram_tensor("wvb", [E, SL], BF16, kind="ExternalInput")
    wpb_d = nc.dram_tensor("wpb", [SL, E], BF16, kind="ExternalInput")
    # aux: cols 0-1 bq (g0,g1), 2-3 bk, 4..259 bv broadcast
    aux_d = nc.dram_tensor("aux", [128, 4 + SL], F32, kind="ExternalInput")
    if CFG["y_direct"]:
        y_d = nc.dram_tensor("y", [N, E], F32, kind="ExternalOutput")
    else:
        y_d = nc.dram_tensor("y", [N, E], BF16, kind="ExternalOutput")

    with TileContext(nc) as tc:
        with (
            tc.tile_pool(name="const", bufs=1) as const,
            tc.tile_pool(name="persist", bufs=1) as persist,
            tc.tile_pool(name="io", bufs=CFG.get("io_bufs", 3)) as io,
            tc.tile_pool(name="small", bufs=CFG.get("small_bufs", 6)) as small,
            tc.tile_pool(name="strips", bufs=CFG.get("strip_bufs", 20)) as strip_pool,
            tc.tile_pool(name="ps_mm", bufs=CFG["ps_mm_bufs"], space="PSUM") as ps_mm,
            tc.tile_pool(name="ps_y", bufs=max(CFG["ps_y_bufs"], 1), space="PSUM") as ps_y,
            tc.tile_pool(name="ps_e", bufs=CFG["ps_e_bufs"], space="PSUM") as ps_e,
            tc.tile_pool(name="ps_u", bufs=CFG["ps_u_bufs"], space="PSUM") as ps_u,
            tc.tile_pool(name="ps_t", bufs=CFG["ps_t_bufs"], space="PSUM") as ps_t,
        ):
            # ---- weights and fp8 x first so q/k compute starts early ----
            x8_sb = persist.tile([128, KO, N], FP8)
            xb_sb = persist.tile([128, KO, N], BF16)
            x8_ap = xT8_d.ap().rearrange("(ko p) n -> p ko n", p=128)
            xb_ap = xTb_d.ap().rearrange("(ko p) n -> p ko n", p=128)
            NCH = 4
            CW = N // NCH  # 512

            aux = const.tile([128, 4 + SL], F32)
            wq_sb = persist.tile([128, KO, SL], FP8)
            nc.sync.dma_start(wq_sb[:], wq8_d.ap().rearrange("(ko p) m -> p ko m", p=128))
            nc.sync.dma_start(x8_sb[:, :, 0:128], x8_ap[:, :, 0:128])
            nc.sync.dma_start(aux[:], aux_d.ap())
            wk_sb = persist.tile([128, KO, SL], FP8)
            nc.sync.dma_start(wk_sb[:], wk8_d.ap().rearrange("(ko p) m -> p ko m", p=128))
            nc.sync.dma_start(x8_sb[:, :, 128:512], x8_ap[:, :, 128:512])
            bq_col = aux[:, 0:2]
            bk_col = aux[:, 2:4]
            bv_bc = aux[:, 4:4 + SL]

            wv_sb = persist.tile([128, KO, SL], BF16)
            wp_sb = persist.tile([128, 2, E], BF16)
            nc.sync.dma_start(wv_sb[:], wvb_d.ap().rearrange("(ko p) m -> p ko m", p=128))

            def xb_piece(c0, c1):  # column pieces of the bf16 x
                s = slice(c0, c1)
                nc.sync.dma_start(xb_sb[:, :, s], xb_ap[:, :, s])

            xb_piece(0, 256)
            nc.sync.dma_start(x8_sb[:, :, 512:1024], x8_ap[:, :, 512:1024])
            xb_piece(256, 512)
            xb_piece(512, 768)
            nc.sync.dma_start(x8_sb[:, :, 1024:1536], x8_ap[:, :, 1024:1536])
            xb_piece(768, 1024)
            xb_piece(1024, 1280)
            nc.sync.dma_start(x8_sb[:, :, 1536:2048], x8_ap[:, :, 1536:2048])
            nc.sync.dma_start(
                wp_sb[:], wpb_d.ap().rearrange("(g p) f -> p g f", p=128))
            xb_piece(1280, 1536)
            xb_piece(1536, 1792)
            xb_piece(1792, 2048)

            # ---- on-chip constants ----
            warm = const.tile([128, 128], BF16)
            nc.gpsimd.memset(warm[:], 0.0)
            ident = const.tile([128, 128], BF16)
            make_identity(nc, ident[:])

            # ---- projection outputs ----
            qsb = persist.tile([128, 2, N], QKD, name="qsb")
            ksb = persist.tile([128, 2, N], QKD, name="ksb")
            q_dr = k_dr = None
            if CFG["energy_dr"]:
                q_dr = [persist.tile([64, 2, N], FP8, name=f"q_dr{i}")
                        for i in range(2)]
                k_dr = [persist.tile([64, 2, N], FP8, name=f"k_dr{i}")
                        for i in range(2)]
            vaug = persist.tile([128, NT, HPC, DH + 1], BF16)
            nc.gpsimd.memset(vaug[:, :, :, DH], 1.0)
            attT = [persist.tile([128, N], BF16, name=f"attT{g}", tag=f"attT{g}")
                    for g in range(2)]

            def emit_qk_chunk(ch, cs=None):
                if cs is None:
                    cs = slice(ch * CW, (ch + 1) * CW)
                w_cs = cs.stop - cs.start
                for w_sb, out_t, b_col in ((wq_sb, qsb, bq_col), (wk_sb, ksb, bk_col)):
                    for g in range(2):
                        ps = ps_mm.tile([128, 512], F32, tag="mm", name="ps_qk")
                        ps = ps[:, :w_cs]
                        for kp in range(KO // 2):
                            nc.tensor.matmul(
                                ps,
                                lhsT=w_sb[:, 2 * kp:2 * kp + 2, g * 128:(g + 1) * 128],
                                rhs=x8_sb[:, 2 * kp:2 * kp + 2, cs],
                                start=(kp == 0), stop=(kp == KO // 2 - 1),
                                perf_mode=DR)
                        if g < CFG.get("qk_act", 1):
                            nc.scalar.activation(
                                out_t[:, g, cs], ps,
                                mybir.ActivationFunctionType.Identity,
                                bias=b_col[:, g:g + 1])
                        else:
                            nc.vector.tensor_scalar_add(
                                out_t[:, g, cs], ps, b_col[:, g:g + 1])

            def emit_v_chunk(ch):
                for nt in range(ch * NCH, (ch + 1) * NCH):
                    emit_v_tile(nt)

            def emit_v_tile(nt):
                if True:
                    ps = ps_mm.tile([128, 512], F32, tag="mm", name="ps_v")
                    psv = ps[:, :SL]
                    rs = slice(nt * 128, (nt + 1) * 128)
                    for kt in range(KO):
                        nc.tensor.matmul(
                            psv, lhsT=xb_sb[:, kt, rs], rhs=wv_sb[:, kt, :],
                            start=(kt == 0), stop=(kt == KO - 1))
                    nc.vector.tensor_add(
                        vaug[:, nt, :, :DH],
                        psv.rearrange("p (h d) -> p h d", d=DH),
                        bv_bc[:].rearrange("p (h d) -> p h d", d=DH))

            # d-split rearrange: qsb/ksb -> q_dr/k_dr for one half of N
            def emit_dsplit(quarter):
                # one DMA per (tensor, head-pair): the source AP splits the
                # partition dim 3 ways (head-in-pair, d-half, d-sub) so both
                # d-halves of both heads move in a single transfer
                hs = slice(quarter * (N // 4), (quarter + 1) * (N // 4))
                for src, dst in ((qsb, q_dr), (ksb, k_dr)):
                    sv = src[:].rearrange("(a w) g n -> a w g n", a=2)
                    for pair in range(2):
                        for i in range(2):
                            nc.sync.dma_start(
                                dst[pair][:, i, hs].rearrange(
                                    "(a w) n -> a w n", a=2),
                                sv[:, 32 * i:32 * i + 32, pair, hs])

            # ---- banded attention ----
            strips = {}

            def emit_strip(h, kj):
                lo, hi = max(0, kj - 1), min(NT - 1, kj + 1)
                w = (hi - lo + 1) * 128
                moff = 0 if lo == kj - 1 else 128
                pe = ps_e.tile([128, 384], F32, tag="pe", name="pe")
                if CFG["dr_direct"]:
                    # q/k already in d-split layout: head h at partitions
                    # 32h..32h+32, dim1 = d-half
                    hb = 32 * h
                    nc.tensor.matmul(
                        pe[:, :w],
                        lhsT=ksb[hb:hb + 32, :, kj * 128:(kj + 1) * 128],
                        rhs=qsb[hb:hb + 32, :, lo * 128:(hi + 1) * 128],
                        start=True, stop=True, perf_mode=DR,
                        tile_position=(hb, 0))
                elif CFG["energy_dr"]:
                    hb = 32 * (h % 2)
                    nc.tensor.matmul(
                        pe[:, :w],
                        lhsT=k_dr[h // 2][hb:hb + 32, :, kj * 128:(kj + 1) * 128],
                        rhs=q_dr[h // 2][hb:hb + 32, :, lo * 128:(hi + 1) * 128],
                        start=True, stop=True, perf_mode=DR)
                else:
                    g, po = h // 2, (h % 2) * 64
                    nc.tensor.matmul(
                        pe[:, :w],
                        lhsT=ksb[po:po + 64, g, kj * 128:(kj + 1) * 128],
                        rhs=qsb[po:po + 64, g, lo * 128:(hi + 1) * 128],
                        start=True, stop=True)
                st = strip_pool.tile([128, 384], BF16, tag="strip", name="st")
                nc.scalar.activation(
                    st[:, :w], pe[:, :w],
                    mybir.ActivationFunctionType.Exp, scale=SCALE)
                # band mask: only the off-diagonal blocks need it (Pool,
                # SBUF-only).  U block: keep c >= p; L block: keep c <= p.
                if lo == kj - 1:
                    nc.gpsimd.affine_select(
                        out=st[:, 0:128], in_=st[:, 0:128],
                        compare_op=mybir.AluOpType.is_ge, fill=0.0, base=0,
                        pattern=[[1, 128]], channel_multiplier=-1)
                if hi == kj + 1:
                    lc = (hi - lo) * 128
                    nc.gpsimd.affine_select(
                        out=st[:, lc:lc + 128], in_=st[:, lc:lc + 128],
                        compare_op=mybir.AluOpType.is_ge, fill=0.0, base=0,
                        pattern=[[-1, 128]], channel_multiplier=1)
                strips[(h, kj)] = (st, lo)

            def stage_att(t):
                ts_ = slice(t * 128, (t + 1) * 128)
                ks = [k for k in (t - 1, t, t + 1) if 0 <= k < NT]
                pu = ps_u.tile([128, HPC, DH + 1], F32, tag="pu", name="pu")
                for h in range(HPC):
                    for i, k2 in enumerate(ks):
                        st, lo2 = strips[(h, k2)]
                        col = (t - lo2) * 128
                        nc.tensor.matmul(
                            pu[:, h, :], lhsT=st[:, col:col + 128],
                            rhs=vaug[:, k2, h, :],
                            start=(i == 0),
                            stop=(i == len(ks) - 1),
                            skip_group_check=True)
                rec = small.tile([128, HPC], F32, tag="rec", name="rec")
                nc.vector.reciprocal(rec[:], pu[:, :, DH])
                ao = small.tile([128, HPC, DH], BF16, tag="ao", name="ao")
                late = t >= CFG.get("late_from", 99)
                if late:
                    for h in range(HPC):
                        nc.scalar.activation(
                            ao[:, h, :], pu[:, h, :DH],
                            mybir.ActivationFunctionType.Identity,
                            scale=rec[:, h:h + 1])
                else:
                    nc.vector.tensor_mul(
                        ao[:], pu[:, :, :DH],
                        rec[:, :, None].broadcast_to([128, HPC, DH]))
                for g in range(2):
                    pt = ps_t.tile([128, 128], BF16, tag="pt", name="pt")
                    nc.tensor.transpose(
                        pt[:], ao[:, 2 * g:2 * g + 2, :], ident[:])
                    att_act = CFG.get("attT_act", 1)  # of 4 copies on Act
                    late_a = late or t >= CFG.get("late_att_from", 99)
                    if (late_a and g == 0) or 2 * (t % 2) + g < att_act:
                        nc.scalar.activation(
                            attT[g][:, ts_], pt[:],
                            mybir.ActivationFunctionType.Copy)
                    else:
                        nc.vector.tensor_copy(attT[g][:, ts_], pt[:])

            ybuf = {}

            def stage_proj(t):
                ts_ = slice(t * 128, (t + 1) * 128)
                if CFG.get("y_pair", False):
                    if t % 2 == 0:
                        ybuf["t"] = io.tile([128, 2, E], BF16, tag="y2", name="y2_sb")
                    y_sb = ybuf["t"][:, t % 2, :]
                else:
                    y_sb = io.tile([128, E], BF16, tag="y", name="y_sb")
                for fc in range(2):
                    if CFG.get("y_in_mm", False):
                        ps = ps_mm.tile([128, 512], F32, tag="mm", name="ps_yt")
                    else:
                        ps = ps_y.tile([128, 512], F32, tag="ymm", name="ps_yt")
                    fs = slice(fc * 512, (fc + 1) * 512)
                    for g in range(2):
                        nc.tensor.matmul(
                            ps[:],
                            lhsT=attT[g][:, ts_],
                            rhs=wp_sb[:, g, fs],
                            start=(g == 0), stop=(g == 1))
                    y_act = CFG.get("y_act", 2)  # of 4 copies on Act
                    late = t >= min(CFG.get("late_from", 99),
                                    CFG.get("late_y_from", 99))
                    if (late and fc == 0) or 2 * (t % 2) + fc < y_act:
                        nc.scalar.activation(
                            y_sb[:, fs], ps[:],
                            mybir.ActivationFunctionType.Copy)
                    else:
                        nc.vector.tensor_copy(y_sb[:, fs], ps[:])
                if CFG.get("y_pair", False):
                    if t % 2 == 1:
                        dst = y_d[(t - 1) * 128:(t + 1) * 128, :]
                        nc.sync.dma_start(
                            dst.rearrange("(tt p) f -> p tt f", p=128),
                            ybuf["t"][:])
                elif CFG.get("y_split", False):
                    nc.sync.dma_start(y_d[ts_, 0:512], y_sb[:, 0:512])
                    nc.sync.dma_start(y_d[ts_, 512:1024], y_sb[:, 512:1024])
                else:
                    nc.sync.dma_start(y_d[ts_, :], y_sb[:])

            # ---- schedule ----
            # PE warmup against the p-state ramp while input DMAs stream
            for i in range(CFG.get("warmups", 12)):
                pw = ps_e.tile([128, 384], F32, tag="pe", name="pe_w")
                nc.tensor.matmul(pw[:, :CFG.get("warm_w", 128)],
                                 lhsT=warm[:],
                                 rhs=warm[:, :CFG.get("warm_w", 128)],
                                 start=True, stop=True)
            emit_qk_chunk(0, cs=slice(0, 128))
            emit_qk_chunk(0, cs=slice(128, 512))
            if CFG["energy_dr"]:
                emit_dsplit(0)
            QKP = CFG.get("qk_pos", (0, 3, 7))
            VL = CFG.get("v_lead", 1)
            for kj in range(NT + CFG.get("proj_lag", 3)):
                if kj < NT:
                    for h in range(HPC):
                        emit_strip(h, kj)
                if kj == QKP[0]:
                    emit_qk_chunk(0, cs=slice(512, 1024))
                    if CFG["energy_dr"]:
                        emit_dsplit(1)
                elif kj == QKP[1]:
                    emit_qk_chunk(0, cs=slice(1024, 1536))
                    if CFG["energy_dr"]:
                        emit_dsplit(2)
                elif kj == QKP[2]:
                    emit_qk_chunk(0, cs=slice(1536, 2048))
                    if CFG["energy_dr"]:
                        emit_dsplit(3)
                if kj == 0:
                    for j in range(VL):
                        emit_v_tile(j)
                if kj + VL < NT:
                    emit_v_tile(kj + VL)
                if CFG.get("tight_tail", True):
                    # steady lag 2/3; once strips end, drain without idle lag
                    if kj < NT:
                        if 2 <= kj < NT - 1:
                            stage_att(kj - 2)
                        elif kj == NT - 1:
                            stage_att(kj - 2)
                            stage_att(kj - 1)
                            stage_att(kj)
                    if 3 <= kj < NT:
                        stage_proj(kj - 3)
                    elif kj == NT:
                        stage_proj(NT - 3)
                        stage_proj(NT - 2)
                        stage_proj(NT - 1)
                else:
                    if 2 <= kj < NT + 2:
                        stage_att(kj - 2)
                    if kj >= CFG.get("proj_lag", 3):
                        stage_proj(kj - CFG.get("proj_lag", 3))

    nc.compile()
    return nc


def _get_nc():
    global _CACHED_NC
    if _CACHED_NC is None:
        _CACHED_NC = _build_nc()
    return _CACHED_NC


def kernel(x, Wq, bq, Wk, bk, Wv, bv, Wp, bp):
    nc = _get_nc()
    x = np.asarray(x, np.float32)
    xTs = [np.ascontiguousarray(x[b].T) for b in range(B)]
    in_maps = []
    for c in range(8):
        b, gq = c // 4, c % 4
        sl = slice(SL * gq, SL * (gq + 1))
        xT = xTs[b]
        aux = np.zeros((128, 4 + SL), np.float32)
        aux[:, 0] = np.asarray(bq, np.float32)[sl][:128]
        aux[:, 1] = np.asarray(bq, np.float32)[sl][128:]
        aux[:, 2] = np.asarray(bk, np.float32)[sl][:128]
        aux[:, 3] = np.asarray(bk, np.float32)[sl][128:]
        aux[:, 4:] = np.asarray(bv, np.float32)[sl][None, :]
        wq_s = np.asarray(Wq, np.float32)[sl]
        wk_s = np.asarray(Wk, np.float32)[sl]
        bq_s = np.asarray(bq, np.float32)[sl]
        bk_s = np.asarray(bk, np.float32)[sl]
        if CFG["dr_direct"]:
            # column j = i*128 + 32h + p  <->  feature h*64 + i*32 + p
            j = np.arange(SL)
            f = (j % 128) // 32 * 64 + (j // 128) * 32 + (j % 32)
            wq_s, wk_s, bq_s, bk_s = wq_s[f], wk_s[f], bq_s[f], bk_s[f]
            aux[:, 0] = bq_s[:128]
            aux[:, 1] = bq_s[128:]
            aux[:, 2] = bk_s[:128]
            aux[:, 3] = bk_s[128:]
        in_maps.append({
            "xT8": xT.astype(NP_FP8),
            "xTb": xT.astype(NP_BF16),
            "wq8": np.ascontiguousarray(wq_s.T).astype(NP_FP8),
            "wk8": np.ascontiguousarray(wk_s.T).astype(NP_FP8),
            "wvb": np.ascontiguousarray(np.asarray(Wv, np.float32)[sl].T).astype(NP_BF16),
            "wpb": np.ascontiguousarray(np.asarray(Wp, np.float32)[:, sl].T).astype(NP_BF16),
            "aux": aux,
        })
    res = run_bass_kernel_spmd(nc, in_maps, core_ids=list(range(8)))
    ys = [np.asarray(res.results[c]["y"], np.float32) for c in range(8)]
    bp = np.asarray(bp, np.float32)
    y = np.stack([
        ys[0] + ys[1] + ys[2] + ys[3],
        ys[4] + ys[5] + ys[6] + ys[7],
    ]).astype(np.float32) + bp[None, None, :]
    return y.astype(np.float32)
